# revision 9
# baseline (speedup 1.0000x reference)
"""Sharded 4-layer GAT kernel for Trainium2 (8 NeuronCores, SPMD).

Strategy (dst-major gather):
- 50176 padded nodes, core c owns dst nodes [c*6272, (c+1)*6272) (49 blocks
  of 128). Edges partitioned by dst owner, grouped per dst-block, split into
  lo/hi src halves (int16 gather index limit), padded to 128-edge tiles with
  a shared (max-over-cores) compile-time schedule.
- Per layer: node phase computes feat|el|er = h @ [W|WAl|WAr] from a
  transposed activation layout hT (fp16 matmuls, f32 psum), writes an fp16
  gather table [feat(256) | el(4) | pad] (768B rows) locally, AllGather to
  a replicated table, plus a local er table (256B rows).
- Edge phase per dst-block: dma_gather of per-edge table rows (by src) and
  er rows (by dst), ex = exp(leakyrelu(el+er)) with the softmax division
  deferred (out_raw and s accumulate in one PSUM via a one-hot matmul per
  128-edge tile: lhsT = O (dst_rel one-hot), rhs = [ex*feat | ex]).
- Tail: r = 1/(s+1e-9), out = relu(r*out_raw + bias) -> next layer h.
  Softmax max-subtraction is skipped (validated: logits <= ~7).
"""
import os
import numpy as np

import concourse.bass as bass
import concourse.mybir as mybir
import concourse.tile as tile
from concourse import bacc
from concourse.bass_utils import run_bass_kernel_spmd

# debug bisect knob: 0=BN only, 1=+node L1+AG, 2=+edge L1, 99=full
_STAGE = int(os.environ.get("GAT_STAGE", "99"))
# edge-phase sub-bisect: gather | ex | full
_ESUB = os.environ.get("GAT_ESUB", "full")

fp16, f32, i16 = mybir.dt.float16, mybir.dt.float32, mybir.dt.int16
AL = mybir.AluOpType

N = 50000
E = 800000
NCORES = 8
P = 128
PER_CORE = 6272            # 49 * 128
NP_PAD = NCORES * PER_CORE  # 50176
NBLK = PER_CORE // P       # 49
SRC_SPLIT = 25088
IN_F, H_F, HEADS, HEAD_D, O_F = 128, 256, 4, 64, 64
ROWW = 384                 # fp16 table row: 256 feat + 4 el + pad  (768B)
ROWW4 = 128                # layer-4 table row: 64 feat + 1 el + pad (256B)
ERW = 128                  # er table row elems (256B)
NEG = 0.2
CH = 7                     # node-phase staging chunk (blocks)


# ---------------------------------------------------------------- host prep
def _preprocess(src, dst):
    src = np.asarray(src).astype(np.int64)
    dst = np.asarray(dst).astype(np.int64)
    core_of = dst // PER_CORE
    blk_of = (dst % PER_CORE) // P
    half = (src >= SRC_SPLIT).astype(np.int64)
    key = ((core_of * NBLK + blk_of) * 2 + half) * N + src
    order = np.argsort(key, kind="stable")
    s_src, s_dst = src[order], dst[order]
    s_core, s_blk, s_half = core_of[order], blk_of[order], half[order]

    cnt = np.zeros((NCORES, NBLK, 2), dtype=np.int64)
    np.add.at(cnt, (s_core, s_blk, s_half), 1)
    tiles = (cnt + 127) // 128
    T_lo = tiles[:, :, 0].max(axis=0)
    T_hi = tiles[:, :, 1].max(axis=0)
    T_blk = T_lo + T_hi
    blk_slot_off = np.zeros(NBLK + 1, dtype=np.int64)
    blk_slot_off[1:] = np.cumsum(T_blk * P)
    total_slots = int(blk_slot_off[-1])

    flat_cnt = cnt.reshape(-1)
    run_off_flat = np.zeros(len(flat_cnt) + 1, dtype=np.int64)
    run_off_flat[1:] = np.cumsum(flat_cnt)
    run_off = run_off_flat[:-1].reshape(NCORES, NBLK, 2)

    per_core = []
    for c in range(NCORES):
        featx_idx = np.zeros(total_slots, dtype=np.int64)
        er_idx = np.zeros(total_slots, dtype=np.int64)
        dst_rel = np.full(total_slots, -1.0, dtype=np.float32)
        for b in range(NBLK):
            off = blk_slot_off[b]
            for h, Th in ((0, T_lo[b]), (1, T_hi[b])):
                n = cnt[c, b, h]
                r0 = run_off[c, b, h]
                base = 0 if h == 0 else SRC_SPLIT
                featx_idx[off:off + n] = s_src[r0:r0 + n] - base
                er_idx[off:off + n] = s_dst[r0:r0 + n] - c * PER_CORE
                dst_rel[off:off + n] = s_dst[r0:r0 + n] - (c * PER_CORE + P * b)
                off += Th * P
        per_core.append((featx_idx, er_idx, dst_rel))

    sched = dict(T_lo=tuple(int(v) for v in T_lo),
                 T_hi=tuple(int(v) for v in T_hi),
                 T_blk=tuple(int(v) for v in T_blk),
                 off=tuple(int(v) for v in blk_slot_off),
                 slots=total_slots)
    return sched, per_core


def _wrap16(idx):
    """int array -> [128, n/16] int16 gather-index layout."""
    n = len(idx)
    flat = idx.astype(np.int16)
    assert n % 16 == 0
    buf = flat.reshape(n // 16, 16).T.copy()
    return np.tile(buf, (8, 1))


# ---------------------------------------------------------------- program
def _build_program(sched):
    T_lo, T_hi, T_blk = sched["T_lo"], sched["T_hi"], sched["T_blk"]
    off = sched["off"]
    SLOTS = sched["slots"]
    TT = SLOTS // P
    TMAX = max(T_blk)

    nc = bacc.Bacc("TRN2", target_bir_lowering=False, debug=False,
                   num_devices=NCORES)

    # inputs
    xs = nc.dram_tensor("xs", [PER_CORE, IN_F], f32, kind="ExternalInput")
    fidx_d = nc.dram_tensor("fidx", [P, SLOTS // 16], i16, kind="ExternalInput")
    drel_d = nc.dram_tensor("drel", [P, TT], fp16, kind="ExternalInput")
    iotab_d = nc.dram_tensor("iotab", [P, 128 * TMAX], fp16, kind="ExternalInput")
    ident_d = nc.dram_tensor("ident", [P, P], fp16, kind="ExternalInput")
    w_d = [nc.dram_tensor("w1", [IN_F, 264], fp16, kind="ExternalInput"),
           nc.dram_tensor("w2", [H_F, 264], fp16, kind="ExternalInput"),
           nc.dram_tensor("w3", [H_F, 264], fp16, kind="ExternalInput"),
           nc.dram_tensor("w4", [H_F, 66], fp16, kind="ExternalInput")]
    b_d = [nc.dram_tensor("b1", [P, 256], f32, kind="ExternalInput"),
           nc.dram_tensor("b2", [P, 256], f32, kind="ExternalInput"),
           nc.dram_tensor("b3", [P, 256], f32, kind="ExternalInput"),
           nc.dram_tensor("b4", [P, 64], f32, kind="ExternalInput")]
    iota_d = nc.dram_tensor("iota", [P, P], fp16, kind="ExternalInput")
    bng_d = nc.dram_tensor("bng", [P, 1], f32, kind="ExternalInput")
    bnb_d = nc.dram_tensor("bnb", [P, 1], f32, kind="ExternalInput")
    out_d = nc.dram_tensor("out", [PER_CORE, O_F], f32, kind="ExternalOutput")

    # internal DRAM
    tloc = [nc.dram_tensor(f"tloc{l}", [PER_CORE, ROWW], fp16) for l in range(3)]
    tloc.append(nc.dram_tensor("tloc3", [PER_CORE, ROWW4], fp16))
    tful = [nc.dram_tensor(f"tful{l}", [NP_PAD, ROWW], fp16, addr_space="Shared")
            for l in range(3)]
    tful.append(nc.dram_tensor("tful3", [NP_PAD, ROWW4], fp16,
                               addr_space="Shared"))
    st_in = nc.dram_tensor("st_in", [P, 2], f32)
    st_out = nc.dram_tensor("st_out", [P, 2], f32, addr_space="Shared")

    RG = [list(range(NCORES))]

    with tile.TileContext(nc) as tc:
        with (
            tc.tile_pool(name="const", bufs=1) as cpool,
            tc.tile_pool(name="big", bufs=2) as bigpool,
            tc.tile_pool(name="ht", bufs=1) as htpool,
            tc.tile_pool(name="h", bufs=1) as hpool,
            tc.tile_pool(name="stage", bufs=2) as stpool,
            tc.tile_pool(name="edge", bufs=2) as epool,
            tc.tile_pool(name="o", bufs=8) as opool,
            tc.tile_pool(name="tail", bufs=2) as tailpool,
            tc.tile_pool(name="npsum", bufs=2, space="PSUM") as npsum,
            tc.tile_pool(name="epsum", bufs=2, space="PSUM") as epsum,
            tc.tile_pool(name="otpsum", bufs=2, space="PSUM") as otpsum,
            tc.tile_pool(name="erpsum", bufs=2, space="PSUM") as erpsum,
        ):
            # ---- constants ----
            drel_sb = cpool.tile([P, TT], fp16)
            nc.sync.dma_start(out=drel_sb[:], in_=drel_d[:, :])
            iota_sb = cpool.tile([P, P], fp16)
            nc.sync.dma_start(out=iota_sb[:], in_=iota_d[:, :])
            iotab_sb = cpool.tile([P, 128 * TMAX], fp16, tag="iotab")
            nc.sync.dma_start(out=iotab_sb[:], in_=iotab_d[:, :])
            ident_sb = cpool.tile([P, P], fp16, tag="ident")
            nc.sync.dma_start(out=ident_sb[:], in_=ident_d[:, :])
            w_sb = []
            for l in range(4):
                kt = 1 if l == 0 else 2
                cols = 66 if l == 3 else 264
                wt = cpool.tile([P, kt, cols], fp16, tag=f"w{l}")
                nc.sync.dma_start(
                    out=wt[:], in_=w_d[l][:, :].rearrange("(a p) c -> p a c", p=P))
                w_sb.append(wt)
            b_sb = []
            for l in range(4):
                cols = 64 if l == 3 else 256
                bt = cpool.tile([P, cols], f32, tag=f"b{l}")
                nc.sync.dma_start(out=bt[:], in_=b_d[l][:, :])
                b_sb.append(bt)
            bng_sb = cpool.tile([P, 1], f32, tag="bng")
            nc.sync.dma_start(out=bng_sb[:], in_=bng_d[:, :])
            bnb_sb = cpool.tile([P, 1], f32, tag="bnb")
            nc.sync.dma_start(out=bnb_sb[:], in_=bnb_d[:, :])

            # ---- BN prologue (transposed layout) ----
            x16 = bigpool.tile([P, NBLK, IN_F], fp16, tag="big")
            nc.gpsimd.dma_start(
                out=x16[:], in_=xs[:, :].rearrange("(m p) c -> p m c", p=P))
            x16T = bigpool.tile([P, NBLK, P], fp16, tag="big")
            nc.sync.dma_start_transpose(x16T[:], x16[:].rearrange("p a b -> p (a b)"))
            xTf = x16T[:].rearrange("p a b -> p (a b)")
            sq = bigpool.tile([P, NBLK * P], fp16, tag="big")
            nc.vector.tensor_tensor(out=sq[:], in0=xTf, in1=xTf, op=AL.mult)
            stats = tailpool.tile([P, 2], f32, tag="stats")
            nc.vector.tensor_reduce(out=stats[:, 0:1], in_=xTf,
                                    axis=mybir.AxisListType.X, op=AL.add)
            nc.vector.tensor_reduce(out=stats[:, 1:2], in_=sq[:],
                                    axis=mybir.AxisListType.X, op=AL.add)
            nc.sync.dma_start(out=st_in[:, :], in_=stats[:])
            nc.gpsimd.collective_compute(
                "AllReduce", AL.add, replica_groups=RG,
                ins=[st_in[:, :]], outs=[st_out[:, :]])
            stat2 = tailpool.tile([P, 2], f32, tag="stats")
            nc.sync.dma_start(out=stat2[:], in_=st_out[:, :])
            mu = tailpool.tile([P, 1], f32, tag="mu")
            nc.vector.tensor_scalar(out=mu[:], in0=stat2[:, 0:1],
                                    scalar1=1.0 / N, scalar2=None, op0=AL.mult)
            var = tailpool.tile([P, 1], f32, tag="var")
            # var = s2/N - mu^2 + eps
            nc.vector.scalar_tensor_tensor(
                out=var[:], in0=stat2[:, 1:2], scalar=1.0 / N,
                in1=mu[:], op0=AL.mult, op1=AL.bypass)
            musq = tailpool.tile([P, 1], f32, tag="musq")
            nc.vector.tensor_tensor(out=musq[:], in0=mu[:], in1=mu[:], op=AL.mult)
            nc.vector.tensor_tensor(out=var[:], in0=var[:], in1=musq[:],
                                    op=AL.subtract)
            nc.vector.tensor_scalar(out=var[:], in0=var[:], scalar1=1e-5,
                                    scalar2=None, op0=AL.add)
            std = tailpool.tile([P, 1], f32, tag="std")
            nc.scalar.activation(out=std[:], in_=var[:],
                                 func=mybir.ActivationFunctionType.Sqrt)
            rstd = tailpool.tile([P, 1], f32, tag="rstd")
            nc.vector.reciprocal(out=rstd[:], in_=std[:])
            gam = tailpool.tile([P, 1], f32, tag="gam")
            nc.vector.tensor_tensor(out=gam[:], in0=bng_sb[:], in1=rstd[:],
                                    op=AL.mult)
            mg = tailpool.tile([P, 1], f32, tag="mg")
            nc.vector.tensor_tensor(out=mg[:], in0=mu[:], in1=gam[:], op=AL.mult)
            bpr = tailpool.tile([P, 1], f32, tag="bpr")
            nc.vector.tensor_tensor(out=bpr[:], in0=bnb_sb[:], in1=mg[:],
                                    op=AL.subtract)
            hT = htpool.tile([P, NBLK, P], fp16, tag="hT")
            nc.vector.tensor_scalar(out=hT[:].rearrange("p a b -> p (a b)"),
                                    in0=xTf, scalar1=gam[:], scalar2=bpr[:],
                                    op0=AL.mult, op1=AL.add)

            # ---- layers ----
            for l in range(4):
                if _STAGE < 1 + 2 * l:
                    break
                KT = 1 if l == 0 else 2
                NCOL = 66 if l == 3 else 264
                TBLW = ROWW4 if l == 3 else ROWW
                FEW = 65 if l == 3 else 260   # feat+el cols
                # node phase
                NHL = 1 if l == 3 else 4
                er_sb = hpool.tile([P, NBLK, NHL], fp16, tag="ersb")
                for chunk in range(NBLK // CH):
                    tstage = stpool.tile([P, CH, TBLW], fp16, tag="tstage")
                    nc.vector.memset(tstage[:], 0.0)
                    for j in range(CH):
                        m = chunk * CH + j
                        pn = npsum.tile([P, NCOL], f32, tag="npsum")
                        for kt in range(KT):
                            nc.tensor.matmul(
                                out=pn[:], lhsT=hT[:, m * KT + kt, :],
                                rhs=w_sb[l][:, kt, :],
                                start=(kt == 0), stop=(kt == KT - 1))
                        nc.vector.tensor_copy(out=tstage[:, j, 0:FEW],
                                              in_=pn[:, 0:FEW])
                        nc.vector.tensor_copy(out=er_sb[:, m, :],
                                              in_=pn[:, FEW:NCOL])
                    rows = slice(chunk * CH * P, (chunk + 1) * CH * P)
                    nc.sync.dma_start(
                        out=tloc[l][rows, :].rearrange("(m p) c -> p m c", p=P),
                        in_=tstage[:])
                nc.gpsimd.collective_compute(
                    "AllGather", AL.bypass, replica_groups=RG,
                    ins=[tloc[l][:, :]], outs=[tful[l][:, :]])
                if _STAGE < 2 + 2 * l:
                    break

                # edge phase
                if l < 3:
                    h_sb = hpool.tile([P, NBLK, H_F], fp16, tag="h")
                for b in range(NBLK):
                    T = T_blk[b]
                    Tl = T_lo[b]
                    o16 = off[b] // 16
                    featg = epool.tile([P, TMAX, TBLW], fp16, tag="featg")
                    rhs = epool.tile([P, TMAX, FEW], fp16, tag="rhs")
                    fi = epool.tile([P, TMAX * 8], i16, tag="fi")
                    nc.sync.dma_start(out=fi[:, 0:T * 8],
                                      in_=fidx_d[:, o16:o16 + T * 8])
                    _gsel = os.environ.get("GAT_GONLY", "all")
                    if Tl and _gsel in ("all", "lo", "lohi"):
                        nc.gpsimd.dma_gather(
                            out_ap=featg[:, 0:Tl, :],
                            in_ap=tful[l][0:SRC_SPLIT, :],
                            idxs_ap=fi[:, 0:Tl * 8],
                            num_idxs=P * Tl, num_idxs_reg=P * Tl,
                            elem_size=TBLW, single_packet=False)
                    if T - Tl and _gsel in ("all", "hi", "lohi"):
                        nc.gpsimd.dma_gather(
                            out_ap=featg[:, Tl:T, :],
                            in_ap=tful[l][SRC_SPLIT:NP_PAD, :],
                            idxs_ap=fi[:, Tl * 8:T * 8],
                            num_idxs=P * (T - Tl), num_idxs_reg=P * (T - Tl),
                            elem_size=TBLW, single_packet=False)
                    if _ESUB == "gather":
                        # consume the gathers cheaply so they aren't dead
                        junk = tailpool.tile([P, 1], f32, tag="junk")
                        nc.vector.tensor_reduce(
                            out=junk[:], in_=featg[:, 0:T, :],
                            axis=mybir.AxisListType.XY, op=AL.max)
                        if l < 3:
                            nc.vector.memset(h_sb[:, b, :], 0.0)
                        else:
                            ostage = stpool.tile([P, O_F], f32, tag="ostage")
                            nc.vector.memset(ostage[:], 0.0)
                            nc.sync.dma_start(
                                out=out_d[b * P:(b + 1) * P, :], in_=ostage[:])
                        continue

                    NH = 1 if l == 3 else 4
                    FC = 64 if l == 3 else 256  # feat cols
                    gt0 = off[b] // P
                    # batched one-hot build for the whole block
                    Ob = epool.tile([P, TMAX, P], fp16, tag="Ob")
                    nc.vector.tensor_tensor(
                        out=Ob[:, 0:T, :],
                        in0=iotab_sb[:, 0:T * P].rearrange(
                            "p (t i) -> p t i", i=P),
                        in1=drel_sb[:, gt0:gt0 + T].to_broadcast([P, T, P]),
                        op=AL.is_equal)
                    # er expand: OT = transpose(O); er_e = OT.T @ er_b
                    erps = erpsum.tile([P, TMAX * NH], f32, tag="erp")
                    for t in range(T):
                        otp = otpsum.tile([P, P], fp16, tag="otp")
                        nc.tensor.transpose(otp[:], Ob[:, t, :], ident_sb[:])
                        ots = opool.tile([P, P], fp16, tag="ots")
                        nc.vector.tensor_copy(out=ots[:], in_=otp[:])
                        nc.tensor.matmul(
                            out=erps[:, t * NH:(t + 1) * NH], lhsT=ots[:],
                            rhs=er_sb[:, b, :], start=True, stop=True)
                    er16 = epool.tile([P, TMAX, NH], fp16, tag="er16")
                    nc.vector.tensor_copy(
                        out=er16[:, 0:T, :],
                        in_=erps[:, 0:T * NH].rearrange("p (t h) -> p t h", h=NH))
                    ebuf = epool.tile([P, TMAX, NH], fp16, tag="ebuf")
                    lrb = epool.tile([P, TMAX, NH], fp16, tag="lrb")
                    nc.vector.tensor_tensor(
                        out=ebuf[:, 0:T, :], in0=featg[:, 0:T, FC:FC + NH],
                        in1=er16[:, 0:T, :], op=AL.add)
                    nc.vector.scalar_tensor_tensor(
                        out=lrb[:, 0:T, :], in0=ebuf[:, 0:T, :], scalar=NEG,
                        in1=ebuf[:, 0:T, :], op0=AL.mult, op1=AL.max)
                    nc.scalar.activation(
                        out=rhs[:, 0:T, FC:FC + NH], in_=lrb[:, 0:T, :],
                        func=mybir.ActivationFunctionType.Exp)
                    nc.vector.tensor_tensor(
                        out=rhs[:, 0:T, 0:FC].rearrange(
                            "p t (h d) -> p t h d", h=NH),
                        in0=featg[:, 0:T, 0:FC].rearrange(
                            "p t (h d) -> p t h d", h=NH),
                        in1=rhs[:, 0:T, FC:FC + NH].to_broadcast(
                            [P, T, NH, 64]),
                        op=AL.mult)
                    pe = epsum.tile([P, FEW], f32, tag="epsum")
                    for t in range(T):
                        nc.tensor.matmul(out=pe[:], lhsT=Ob[:, t, :],
                                         rhs=rhs[:, t, :],
                                         start=(t == 0), stop=(t == T - 1))
                    seps = tailpool.tile([P, NH], f32, tag="seps")
                    nc.vector.tensor_scalar(out=seps[:], in0=pe[:, FC:FC + NH],
                                            scalar1=1e-9, scalar2=None,
                                            op0=AL.add)
                    rr = tailpool.tile([P, NH], f32, tag="rr")
                    nc.vector.reciprocal(out=rr[:], in_=seps[:])
                    if l < 3:
                        tmp = tailpool.tile([P, 256], f32, tag="tmp")
                        for h in range(4):
                            nc.vector.scalar_tensor_tensor(
                                out=tmp[:, h * 64:(h + 1) * 64],
                                in0=pe[:, h * 64:(h + 1) * 64],
                                scalar=rr[:, h:h + 1],
                                in1=b_sb[l][:, h * 64:(h + 1) * 64],
                                op0=AL.mult, op1=AL.add)
                        nc.vector.tensor_scalar(
                            out=h_sb[:, b, :], in0=tmp[:], scalar1=0.0,
                            scalar2=None, op0=AL.max)
                    else:
                        ostage = stpool.tile([P, O_F], f32, tag="ostage")
                        nc.vector.scalar_tensor_tensor(
                            out=ostage[:], in0=pe[:, 0:64],
                            scalar=rr[:, 0:1], in1=b_sb[3][:, :],
                            op0=AL.mult, op1=AL.add)
                        nc.sync.dma_start(
                            out=out_d[b * P:(b + 1) * P, :],
                            in_=ostage[:])
                if l < 3:
                    hT = htpool.tile([P, 2 * NBLK, P], fp16, tag="hT")
                    nc.sync.dma_start_transpose(
                        hT[:], h_sb[:].rearrange("p a b -> p (a b)"))
    nc.compile()
    return nc


_PROGRAM_CACHE = {}


def _get_program(sched):
    key = (sched["T_lo"], sched["T_hi"])
    if key not in _PROGRAM_CACHE:
        _PROGRAM_CACHE[key] = _build_program(sched)
    return _PROGRAM_CACHE[key]


def _host_inputs(inputs, sched, per_core):
    x = np.asarray(inputs["x"], np.float32)
    xp = np.zeros((NP_PAD, IN_F), np.float32)
    xp[:N] = x

    def catw(W, al, ar, ncol):
        W = np.asarray(W, np.float32)
        al = np.asarray(al, np.float32)[0]  # [H, D]
        ar = np.asarray(ar, np.float32)[0]
        H, D = al.shape
        out = np.zeros((W.shape[0], ncol), np.float32)
        out[:, :H * D] = W
        for h in range(H):
            out[:, H * D + h] = W[:, h * D:(h + 1) * D] @ al[h]
            out[:, H * D + H + h] = W[:, h * D:(h + 1) * D] @ ar[h]
        return out.astype(np.float16)

    w1 = catw(inputs["W1"], inputs["al1"], inputs["ar1"], 264)
    w2 = catw(inputs["W2"], inputs["al2"], inputs["ar2"], 264)
    w3 = catw(inputs["W3"], inputs["al3"], inputs["ar3"], 264)
    w4 = catw(inputs["W4"], inputs["al4"], inputs["ar4"], 66)
    b1 = np.tile(np.asarray(inputs["b1"], np.float32), (P, 1))
    b2 = np.tile(np.asarray(inputs["b2"], np.float32), (P, 1))
    b3 = np.tile(np.asarray(inputs["b3"], np.float32), (P, 1))
    b4 = np.tile(np.asarray(inputs["b4"], np.float32), (P, 1))
    iota = np.tile(np.arange(P, dtype=np.float16), (P, 1))
    TMAX = max(sched["T_blk"])
    iotab = np.tile(np.arange(P, dtype=np.float16), (P, TMAX))
    ident = np.eye(P, dtype=np.float16)
    bng = np.asarray(inputs["bn_gamma"], np.float32).reshape(P, 1)
    bnb = np.asarray(inputs["bn_beta"], np.float32).reshape(P, 1)

    in_maps = []
    TT = sched["slots"] // P
    for c in range(NCORES):
        fi, ei, dr = per_core[c]
        drel = dr.reshape(TT, P).T.copy().astype(np.float16)  # [128, TT]
        in_maps.append({
            "xs": xp[c * PER_CORE:(c + 1) * PER_CORE].copy(),
            "fidx": _wrap16(fi), "drel": drel,
            "iotab": iotab, "ident": ident,
            "w1": w1, "w2": w2, "w3": w3, "w4": w4,
            "b1": b1, "b2": b2, "b3": b3, "b4": b4,
            "iota": iota, "bng": bng, "bnb": bnb,
        })
    return in_maps


def kernel(**inputs):
    sched, per_core = _preprocess(inputs["src"], inputs["dst"])
    nc = _get_program(sched)
    in_maps = _host_inputs(inputs, sched, per_core)
    res = run_bass_kernel_spmd(nc, in_maps, core_ids=list(range(NCORES)))
    out = np.concatenate([res.results[c]["out"] for c in range(NCORES)], axis=0)
    return np.ascontiguousarray(out[:N]).astype(np.float32)
